# revision 18
# baseline (speedup 1.0000x reference)
"""Trainium2 Bass kernel for DocREModel_KD head (ragged_sequence).

Problem shape (hardcoded, per spec):
  sequence_output [4, 1024, 768] f32
  attention       [4, 12, 1024, 1024] f32
  entity_starts   [4, 42, 4] int
  hts             [4, 1764, 2] int
Outputs: (hss, rss, tss) each [4, 42, 42, 768] f32.

Strategy (8 cores, SPMD single program):
  - 2 cores per document. Core parity rho in {0,1}: the pair grid over the
    42x42 entity pairs is split by head-entity. The program always computes
    grid rows i in [0..20]; core rho=1 receives entity indices rotated by 21
    (host-side permutation of the tiny index tensors), so the same program
    computes the other half of the grid.
  - All data-dependent gathers use indirect DMA with host-computed row
    indices fed as int32 inputs (SPMD-safe).
  - e_att (mention-mean of attention rows) is computed c-partitioned by a
    single fused PE matmul against a constant selection matrix (mean +
    transpose in one step, f32r).
  - Pair grid G[c,(i,j)] = sum_h EA[c,i,h]*EA[c,j,h] via one broadcast-AP
    DVE product (bf16, 2x mode) + grouped tree reduction, relu'd.
  - rs = (relu(G) @ seq_aug) with an appended ones column giving the
    normalizer for free; normalization folded into the PSUM drain.
  - e_emb logsumexp is d-split across the core pair (rho chooses which half
    of the hidden dim), exp/ln on ScalarE.
  - hss/tss (pure row replications of e_emb) and the hts->grid-row mapping
    are assembled host-side from the device-computed e_emb / rs grid.
"""

import numpy as np
from contextlib import ExitStack

import concourse.bass as bass
import concourse.bacc as bacc
import concourse.mybir as mybir
import concourse.tile as tile
from concourse.bass_utils import run_bass_kernel_spmd

# ---- problem constants ----
B, H, C, HS, NE, M = 4, 12, 1024, 768, 42, 4
OFFSET = 1
NH = NE * H          # 504 (n,h) pairs
IL = NE // 2         # 21 grid rows per core
U = IL * NE          # 882 pairs per core
PPT = 126            # partitions per gathered RAW tile (504 = 4*126)
NCH = C // 128       # 8 c-chunks
WLSE = HS // 2       # 384: e_emb d-split width per core
N_CORES = 8

F32 = mybir.dt.float32
F32R = mybir.dt.float32r
BF16 = mybir.dt.bfloat16
I32 = mybir.dt.int32

_prog_cache = {}


def _build_program():
    nc = bacc.Bacc(None)

    att = nc.dram_tensor("att", [H * C, C], F32R, kind="ExternalInput")
    seq = nc.dram_tensor("seq", [C, HS], F32, kind="ExternalInput")
    seq_lse = nc.dram_tensor("seq_lse", [C, WLSE], F32, kind="ExternalInput")
    sel_d = nc.dram_tensor("sel", [PPT, 4 * NH], F32R, kind="ExternalInput")
    idx_att_d = nc.dram_tensor("idx_att", [PPT, 16], I32, kind="ExternalInput")
    idx_seq_d = nc.dram_tensor("idx_seq", [NE, M], I32, kind="ExternalInput")

    rs_out = nc.dram_tensor("rs_out", [U, HS], F32, kind="ExternalOutput")
    eemb_out = nc.dram_tensor("eemb_out", [NE, WLSE], F32, kind="ExternalOutput")

    with tile.TileContext(nc) as tc, ExitStack() as ctx:
        const_p = ctx.enter_context(tc.tile_pool(name="const", bufs=1))
        raw_p = ctx.enter_context(tc.tile_pool(name="raw", bufs=1))
        seqf_p = ctx.enter_context(tc.tile_pool(name="seqf", bufs=1))
        seqb_p = ctx.enter_context(tc.tile_pool(name="seqb", bufs=1))
        ea_p = ctx.enter_context(tc.tile_pool(name="ea", bufs=1))
        pr_p = ctx.enter_context(tc.tile_pool(name="pr", bufs=1))
        t4_p = ctx.enter_context(tc.tile_pool(name="t4", bufs=1))
        t2_p = ctx.enter_context(tc.tile_pool(name="t2", bufs=1))
        g_p = ctx.enter_context(tc.tile_pool(name="g", bufs=1))
        lse_p = ctx.enter_context(tc.tile_pool(name="lse", bufs=1))
        rst_p = ctx.enter_context(tc.tile_pool(name="rst", bufs=3))
        small_p = ctx.enter_context(tc.tile_pool(name="small", bufs=2))

        ea_ps = ctx.enter_context(tc.tile_pool(name="eaps", bufs=2, space="PSUM"))
        rsA_ps = ctx.enter_context(tc.tile_pool(name="rsA", bufs=3, space="PSUM"))
        rsB_ps = ctx.enter_context(tc.tile_pool(name="rsB", bufs=3, space="PSUM"))

        # --- constants / indices to SBUF ---
        sel_sb = const_p.tile([PPT, 4 * NH], F32R, name="sel_sb")
        nc.sync.dma_start(out=sel_sb[:], in_=sel_d[:])
        ia_sb = const_p.tile([PPT, 16], I32, name="ia_sb")
        nc.sync.dma_start(out=ia_sb[:], in_=idx_att_d[:])
        is_sb = const_p.tile([NE, M], I32, name="is_sb")
        nc.sync.dma_start(out=is_sb[:], in_=idx_seq_d[:])

        # --- indirect gathers: attention mention rows, mention-summed in
        # flight via the DMA CCE (compute_op=add) so only 4 RAW tiles and
        # 4 SEL matmuls per chunk are needed ---
        raws = []
        for q in range(4):
            raws.append(raw_p.tile([PPT, C], F32R, name=f"raw{q}"))
        for m in range(4):
            for q in range(4):
                t = m * 4 + q
                nc.gpsimd.indirect_dma_start(
                    out=raws[q][:],
                    out_offset=None,
                    in_=att[:],
                    in_offset=bass.IndirectOffsetOnAxis(ap=ia_sb[:, t : t + 1], axis=0),
                    compute_op=(mybir.AluOpType.bypass if m == 0 else mybir.AluOpType.add),
                )

        # --- e_emb logsumexp pipeline (d-split half, exact fp32) ---
        sg = []
        for r in range(M):
            g = lse_p.tile([NE, WLSE], F32, name=f"sg{r}")
            nc.gpsimd.indirect_dma_start(
                out=g[:],
                out_offset=None,
                in_=seq_lse[:],
                in_offset=bass.IndirectOffsetOnAxis(ap=is_sb[:, r : r + 1], axis=0),
            )
            sg.append(g)
        ex = []
        for r in range(M):
            e = lse_p.tile([NE, WLSE], F32, name=f"ex{r}")
            nc.scalar.activation(out=e[:], in_=sg[r][:], func=mybir.ActivationFunctionType.Exp)
            ex.append(e)
        s01 = lse_p.tile([NE, WLSE], F32, name="s01")
        s23 = lse_p.tile([NE, WLSE], F32, name="s23")
        nc.vector.tensor_add(out=s01[:], in0=ex[0][:], in1=ex[1][:])
        nc.vector.tensor_add(out=s23[:], in0=ex[2][:], in1=ex[3][:])
        nc.vector.tensor_add(out=s01[:], in0=s01[:], in1=s23[:])
        lse_res = lse_p.tile([NE, WLSE], F32, name="lse_res")
        nc.scalar.activation(out=lse_res[:], in_=s01[:], func=mybir.ActivationFunctionType.Ln)
        # ACT-issued DMA: same-engine ordering after the Ln, so the DMA
        # carries only its ring-FIFO wait.
        nc.scalar.dma_start(out=eemb_out[:], in_=lse_res[:])

        # --- sequence chunks: load f32, convert to bf16, append ones col ---
        seqb = []
        for k in range(NCH):
            sf = seqf_p.tile([128, HS], F32, name=f"sf{k}")
            nc.sync.dma_start(out=sf[:], in_=seq[k * 128 : (k + 1) * 128, :])
            sb = seqb_p.tile([128, HS + 1], BF16, name=f"sb{k}")
            nc.scalar.copy(out=sb[:, 0:HS], in_=sf[:])
            nc.vector.memset(sb[:, HS : HS + 1], 1.0)
            seqb.append(sb)

        # --- EA: mention-mean + transpose via SEL matmul (f32r) ---
        eas = []
        for k in range(NCH):
            ps = ea_ps.tile([128, NH], F32, name="eaps")
            for q in range(4):
                nc.tensor.matmul(
                    out=ps[:],
                    lhsT=raws[q][:, k * 128 : (k + 1) * 128],
                    rhs=sel_sb[:, q * NH : (q + 1) * NH],
                    start=(q == 0),
                    stop=(q == 3),
                )
            ea = ea_p.tile([128, NH], BF16, name=f"ea{k}")
            nc.scalar.copy(out=ea[:], in_=ps[:])
            eas.append(ea)

        # --- pair-grid products + grouped h-reduction + relu ---
        gs = []
        for k in range(NCH):
            pr = pr_p.tile([128, U * H], BF16, name="pr")
            ea3 = eas[k][:].rearrange("p (i h) -> p i h", h=H)          # [128, 42, 12]
            in0 = ea3[:, 0:IL, :].unsqueeze(2).to_broadcast([128, IL, NE, H])
            in1 = ea3.unsqueeze(1).to_broadcast([128, IL, NE, H])
            pr4 = pr[:].rearrange("p (i j h) -> p i j h", j=NE, h=H)
            nc.vector.tensor_tensor(out=pr4, in0=in0, in1=in1, op=mybir.AluOpType.mult)

            pru = pr[:].rearrange("p (u h) -> p u h", h=H)              # [128, U, 12]
            t4 = t4_p.tile([128, U * 4], BF16, name="t4")
            t4v = t4[:].rearrange("p (u f) -> p u f", f=4)
            nc.vector.tensor_tensor(out=t4v, in0=pru[:, :, 0:4], in1=pru[:, :, 4:8], op=mybir.AluOpType.add)
            nc.vector.tensor_tensor(out=t4v, in0=t4v, in1=pru[:, :, 8:12], op=mybir.AluOpType.add)
            t2 = t2_p.tile([128, U * 2], BF16, name="t2")
            t2v = t2[:].rearrange("p (u f) -> p u f", f=2)
            nc.vector.tensor_tensor(out=t2v, in0=t4v[:, :, 0:2], in1=t4v[:, :, 2:4], op=mybir.AluOpType.add)
            gp = g_p.tile([128, U], BF16, name=f"gp{k}")
            a = t2v[:, :, 0:1].squeeze(2)
            b = t2v[:, :, 1:2].squeeze(2)
            nc.vector.tensor_tensor(out=gp[:], in0=a, in1=b, op=mybir.AluOpType.add)
            g_t = g_p.tile([128, U], BF16, name=f"g{k}")
            nc.scalar.activation(out=g_t[:], in_=gp[:], func=mybir.ActivationFunctionType.Relu)
            gs.append(g_t)

        # --- rs matmul + fused normalization drain (drains on ACT so the
        # ACT-issued output DMA and the PSUM-bank-reuse waits stay single) ---
        sts = []
        for tau in range(7):
            psA = rsA_ps.tile([PPT, 512], F32, name="psA")
            psB = rsB_ps.tile([PPT, HS + 1 - 512], F32, name="psB")   # [126, 257]
            lo = tau * PPT
            for k in range(NCH):
                nc.tensor.matmul(
                    out=psA[:],
                    lhsT=gs[k][:, lo : lo + PPT],
                    rhs=seqb[k][:, 0:512],
                    start=(k == 0),
                    stop=(k == NCH - 1),
                )
            for k in range(NCH):
                nc.tensor.matmul(
                    out=psB[:],
                    lhsT=gs[k][:, lo : lo + PPT],
                    rhs=seqb[k][:, 512 : HS + 1],
                    start=(k == 0),
                    stop=(k == NCH - 1),
                )
            dsum = small_p.tile([PPT, 1], F32, name="dsum")
            nc.vector.tensor_scalar_add(out=dsum[:], in0=psB[:, 256:257], scalar1=1e-10)
            drec = small_p.tile([PPT, 1], F32, name="drec")
            nc.vector.reciprocal(out=drec[:], in_=dsum[:])
            st = rst_p.tile([PPT, HS], F32, name="st")
            nc.vector.tensor_scalar_mul(out=st[:, 0:512], in0=psA[:], scalar1=drec[:])
            nc.scalar.activation(
                out=st[:, 512:HS], in_=psB[:, 0:256],
                func=mybir.ActivationFunctionType.Copy, scale=drec[:],
            )
            nc.sync.dma_start(out=rs_out[lo : lo + PPT, :], in_=st[:])
            sts.append(st)

    nc.finalize()
    return nc


def _host_inputs(sequence_output, attention, entity_starts):
    """Build the 8 per-core input maps."""
    sel_np = np.zeros([PPT, 4 * NH], np.float32)
    for q in range(4):
        for p in range(PPT):
            sel_np[p, q * NH + q * PPT + p] = 0.25

    in_maps = []
    for cid in range(N_CORES):
        d, rho = cid // 2, cid % 2
        perm = (np.arange(NE) + rho * IL) % NE            # slot -> real entity
        starts_doc = np.asarray(entity_starts[d], dtype=np.int64)
        pstarts = starts_doc[perm]                        # [42, 4]
        pos = pstarts + OFFSET                            # mention positions, < 1024

        ia = np.zeros([PPT, 16], np.int32)
        for t in range(16):
            r, q = t // 4, t % 4
            p = np.arange(PPT)
            g = q * PPT + p
            n, h = g // H, g % H
            ia[:, t] = (h * C + pos[n, r]).astype(np.int32)

        iseq = pos.astype(np.int32)                       # [42, 4]

        att_doc = np.ascontiguousarray(
            np.asarray(attention[d], dtype=np.float32).reshape(H * C, C)
        )
        seq_doc = np.ascontiguousarray(np.asarray(sequence_output[d], dtype=np.float32))
        seq_lse = np.ascontiguousarray(seq_doc[:, rho * WLSE : (rho + 1) * WLSE])

        in_maps.append(
            {
                "att": att_doc,
                "seq": seq_doc,
                "seq_lse": seq_lse,
                "sel": sel_np,
                "idx_att": ia,
                "idx_seq": iseq,
            }
        )
    return in_maps


def _assemble(results, entity_starts, hts):
    eemb = np.empty([B, NE, HS], np.float32)
    rs_grid = np.empty([B, NE, NE, HS], np.float32)
    perm1 = (np.arange(NE) + IL) % NE
    for d in range(B):
        o0 = results[2 * d]["eemb_out"]
        o1 = results[2 * d + 1]["eemb_out"]
        eemb[d, :, 0:WLSE] = o0
        eemb[d, perm1, WLSE:HS] = o1

        g0 = results[2 * d]["rs_out"].reshape(IL, NE, HS)
        g1 = results[2 * d + 1]["rs_out"].reshape(IL, NE, HS)
        rs_grid[d, 0:IL] = g0
        tmp = np.empty([IL, NE, HS], np.float32)
        tmp[:, perm1] = g1
        rs_grid[d, IL:NE] = tmp

    hts_np = np.asarray(hts, dtype=np.int64)
    h_idx = hts_np[:, :, 0]
    t_idx = hts_np[:, :, 1]
    hss = np.empty([B, NE * NE, HS], np.float32)
    rss = np.empty([B, NE * NE, HS], np.float32)
    tss = np.empty([B, NE * NE, HS], np.float32)
    for d in range(B):
        hss[d] = eemb[d][h_idx[d]]
        tss[d] = eemb[d][t_idx[d]]
        rss[d] = rs_grid[d][h_idx[d], t_idx[d]]
    shape = (B, NE, NE, HS)
    return hss.reshape(shape), rss.reshape(shape), tss.reshape(shape)


def kernel(sequence_output, attention, entity_starts, hts):
    if "nc" not in _prog_cache:
        _prog_cache["nc"] = _build_program()
    nc = _prog_cache["nc"]

    in_maps = _host_inputs(sequence_output, attention, entity_starts)
    res = run_bass_kernel_spmd(nc, in_maps, list(range(N_CORES))).results
    return _assemble(res, entity_starts, hts)


if __name__ == "__main__":
    # smoke test with random data
    rng = np.random.default_rng(0)
    seq = rng.standard_normal((B, C, HS), dtype=np.float32)
    att = rng.random((B, H, C, C), dtype=np.float32)
    starts = rng.integers(0, 1020, (B, NE, M))
    hts = rng.integers(0, NE, (B, NE * NE, 2))
    outs = kernel(seq, att, starts, hts)
    print([o.shape for o in outs])


# revision 19
# speedup vs baseline: 1.0266x; 1.0266x over previous
"""Trainium2 Bass kernel for DocREModel_KD head (ragged_sequence).

Problem shape (hardcoded, per spec):
  sequence_output [4, 1024, 768] f32
  attention       [4, 12, 1024, 1024] f32
  entity_starts   [4, 42, 4] int
  hts             [4, 1764, 2] int
Outputs: (hss, rss, tss) each [4, 42, 42, 768] f32.

Strategy (8 cores, SPMD single program):
  - 2 cores per document. Core parity rho in {0,1}: the pair grid over the
    42x42 entity pairs is split by head-entity. The program always computes
    grid rows i in [0..20]; core rho=1 receives entity indices rotated by 21
    (host-side permutation of the tiny index tensors), so the same program
    computes the other half of the grid.
  - All data-dependent gathers use indirect DMA with host-computed row
    indices fed as int32 inputs (SPMD-safe).
  - e_att (mention-mean of attention rows) is computed c-partitioned by a
    single fused PE matmul against a constant selection matrix (mean +
    transpose in one step, f32r).
  - Pair grid G[c,(i,j)] = sum_h EA[c,i,h]*EA[c,j,h] via one broadcast-AP
    DVE product (bf16, 2x mode) + grouped tree reduction, relu'd.
  - rs = (relu(G) @ seq_aug) with an appended ones column giving the
    normalizer for free; normalization folded into the PSUM drain.
  - e_emb logsumexp is d-split across the core pair (rho chooses which half
    of the hidden dim), exp/ln on ScalarE.
  - hss/tss (pure row replications of e_emb) and the hts->grid-row mapping
    are assembled host-side from the device-computed e_emb / rs grid.
"""

import numpy as np
from contextlib import ExitStack

import concourse.bass as bass
import concourse.bacc as bacc
import concourse.mybir as mybir
import concourse.tile as tile
from concourse.bass_utils import run_bass_kernel_spmd

# ---- problem constants ----
B, H, C, HS, NE, M = 4, 12, 1024, 768, 42, 4
OFFSET = 1
NH = NE * H          # 504 (n,h) pairs
IL = NE // 2         # 21 grid rows per core
U = IL * NE          # 882 pairs per core
PPT = 126            # partitions per gathered RAW tile (504 = 4*126)
NCH = C // 128       # 8 c-chunks
WLSE = HS // 2       # 384: e_emb d-split width per core
N_CORES = 8

F32 = mybir.dt.float32
F32R = mybir.dt.float32r
BF16 = mybir.dt.bfloat16
I32 = mybir.dt.int32

_prog_cache = {}


def _build_program():
    nc = bacc.Bacc(None)

    att = nc.dram_tensor("att", [H * C, C], F32R, kind="ExternalInput")
    seq = nc.dram_tensor("seq", [C, HS], F32, kind="ExternalInput")
    seq_lse = nc.dram_tensor("seq_lse", [C, WLSE], F32, kind="ExternalInput")
    sel_d = nc.dram_tensor("sel", [PPT, 4 * NH], F32R, kind="ExternalInput")
    idx_att_d = nc.dram_tensor("idx_att", [PPT, 16], I32, kind="ExternalInput")
    idx_seq_d = nc.dram_tensor("idx_seq", [NE, M], I32, kind="ExternalInput")

    rs_out = nc.dram_tensor("rs_out", [U, HS], F32, kind="ExternalOutput")
    eemb_out = nc.dram_tensor("eemb_out", [NE, WLSE], F32, kind="ExternalOutput")

    with tile.TileContext(nc) as tc, ExitStack() as ctx:
        const_p = ctx.enter_context(tc.tile_pool(name="const", bufs=1))
        raw_p = ctx.enter_context(tc.tile_pool(name="raw", bufs=1))
        seqf_p = ctx.enter_context(tc.tile_pool(name="seqf", bufs=1))
        seqb_p = ctx.enter_context(tc.tile_pool(name="seqb", bufs=1))
        ea_p = ctx.enter_context(tc.tile_pool(name="ea", bufs=1))
        pr_p = ctx.enter_context(tc.tile_pool(name="pr", bufs=1))
        t4_p = ctx.enter_context(tc.tile_pool(name="t4", bufs=1))
        t2_p = ctx.enter_context(tc.tile_pool(name="t2", bufs=1))
        g_p = ctx.enter_context(tc.tile_pool(name="g", bufs=1))
        lse_p = ctx.enter_context(tc.tile_pool(name="lse", bufs=1))
        rst_p = ctx.enter_context(tc.tile_pool(name="rst", bufs=2))
        small_p = ctx.enter_context(tc.tile_pool(name="small", bufs=2))

        ea_ps = ctx.enter_context(tc.tile_pool(name="eaps", bufs=2, space="PSUM"))
        rsA_ps = ctx.enter_context(tc.tile_pool(name="rsA", bufs=3, space="PSUM"))
        rsB_ps = ctx.enter_context(tc.tile_pool(name="rsB", bufs=3, space="PSUM"))

        # --- constants / indices to SBUF ---
        sel_sb = const_p.tile([PPT, 4 * NH], F32R, name="sel_sb")
        nc.sync.dma_start(out=sel_sb[:], in_=sel_d[:])
        ia_sb = const_p.tile([PPT, 16], I32, name="ia_sb")
        nc.sync.dma_start(out=ia_sb[:], in_=idx_att_d[:])
        is_sb = const_p.tile([NE, M], I32, name="is_sb")
        nc.sync.dma_start(out=is_sb[:], in_=idx_seq_d[:])

        # --- indirect gathers: attention mention rows ---
        raws = []
        for t in range(16):
            rt = raw_p.tile([PPT, C], F32R, name=f"raw{t}")
            nc.gpsimd.indirect_dma_start(
                out=rt[:],
                out_offset=None,
                in_=att[:],
                in_offset=bass.IndirectOffsetOnAxis(ap=ia_sb[:, t : t + 1], axis=0),
            )
            raws.append(rt)

        # --- e_emb logsumexp pipeline (d-split half, exact fp32) ---
        sg = []
        for r in range(M):
            g = lse_p.tile([NE, WLSE], F32, name=f"sg{r}")
            nc.gpsimd.indirect_dma_start(
                out=g[:],
                out_offset=None,
                in_=seq_lse[:],
                in_offset=bass.IndirectOffsetOnAxis(ap=is_sb[:, r : r + 1], axis=0),
            )
            sg.append(g)
        ex = []
        for r in range(M):
            e = lse_p.tile([NE, WLSE], F32, name=f"ex{r}")
            nc.scalar.activation(out=e[:], in_=sg[r][:], func=mybir.ActivationFunctionType.Exp)
            ex.append(e)
        s01 = lse_p.tile([NE, WLSE], F32, name="s01")
        s23 = lse_p.tile([NE, WLSE], F32, name="s23")
        nc.vector.tensor_add(out=s01[:], in0=ex[0][:], in1=ex[1][:])
        nc.vector.tensor_add(out=s23[:], in0=ex[2][:], in1=ex[3][:])
        nc.vector.tensor_add(out=s01[:], in0=s01[:], in1=s23[:])
        lse_res = lse_p.tile([NE, WLSE], F32, name="lse_res")
        nc.scalar.activation(out=lse_res[:], in_=s01[:], func=mybir.ActivationFunctionType.Ln)
        # ACT-issued DMA: same-engine ordering after the Ln, so the DMA
        # carries only its ring-FIFO wait.
        nc.scalar.dma_start(out=eemb_out[:], in_=lse_res[:])

        # --- sequence chunks: load f32, convert to bf16, append ones col ---
        seqb = []
        for k in range(NCH):
            sf = seqf_p.tile([128, HS], F32, name=f"sf{k}")
            nc.sync.dma_start(out=sf[:], in_=seq[k * 128 : (k + 1) * 128, :])
            sb = seqb_p.tile([128, HS + 1], BF16, name=f"sb{k}")
            nc.scalar.copy(out=sb[:, 0:HS], in_=sf[:])
            nc.vector.memset(sb[:, HS : HS + 1], 1.0)
            seqb.append(sb)

        # --- EA: mention-mean + transpose via SEL matmul (f32r) ---
        eas = []
        for k in range(NCH):
            ps = ea_ps.tile([128, NH], F32, name="eaps")
            for t in range(16):
                q = t % 4
                nc.tensor.matmul(
                    out=ps[:],
                    lhsT=raws[t][:, k * 128 : (k + 1) * 128],
                    rhs=sel_sb[:, q * NH : (q + 1) * NH],
                    start=(t == 0),
                    stop=(t == 15),
                )
            ea = ea_p.tile([128, NH], BF16, name=f"ea{k}")
            nc.scalar.copy(out=ea[:], in_=ps[:])
            eas.append(ea)

        # --- pair-grid products + grouped h-reduction + relu ---
        gs = []
        for k in range(NCH):
            pr = pr_p.tile([128, U * H], BF16, name="pr")
            ea3 = eas[k][:].rearrange("p (i h) -> p i h", h=H)          # [128, 42, 12]
            in0 = ea3[:, 0:IL, :].unsqueeze(2).to_broadcast([128, IL, NE, H])
            in1 = ea3.unsqueeze(1).to_broadcast([128, IL, NE, H])
            pr4 = pr[:].rearrange("p (i j h) -> p i j h", j=NE, h=H)
            nc.vector.tensor_tensor(out=pr4, in0=in0, in1=in1, op=mybir.AluOpType.mult)

            pru = pr[:].rearrange("p (u h) -> p u h", h=H)              # [128, U, 12]
            t4 = t4_p.tile([128, U * 4], BF16, name="t4")
            t4v = t4[:].rearrange("p (u f) -> p u f", f=4)
            nc.vector.tensor_tensor(out=t4v, in0=pru[:, :, 0:4], in1=pru[:, :, 4:8], op=mybir.AluOpType.add)
            nc.vector.tensor_tensor(out=t4v, in0=t4v, in1=pru[:, :, 8:12], op=mybir.AluOpType.add)
            t2 = t2_p.tile([128, U * 2], BF16, name="t2")
            t2v = t2[:].rearrange("p (u f) -> p u f", f=2)
            nc.vector.tensor_tensor(out=t2v, in0=t4v[:, :, 0:2], in1=t4v[:, :, 2:4], op=mybir.AluOpType.add)
            gp = g_p.tile([128, U], BF16, name=f"gp{k}")
            a = t2v[:, :, 0:1].squeeze(2)
            b = t2v[:, :, 1:2].squeeze(2)
            nc.vector.tensor_tensor(out=gp[:], in0=a, in1=b, op=mybir.AluOpType.add)
            g_t = g_p.tile([128, U], BF16, name=f"g{k}")
            nc.scalar.activation(out=g_t[:], in_=gp[:], func=mybir.ActivationFunctionType.Relu)
            gs.append(g_t)

        # --- rs matmul + fused normalization drain (drains on ACT so the
        # ACT-issued output DMA and the PSUM-bank-reuse waits stay single) ---
        sts = []
        for tau in range(7):
            psA = rsA_ps.tile([PPT, 512], F32, name="psA")
            psB = rsB_ps.tile([PPT, HS + 1 - 512], F32, name="psB")   # [126, 257]
            lo = tau * PPT
            for k in range(NCH):
                nc.tensor.matmul(
                    out=psA[:],
                    lhsT=gs[k][:, lo : lo + PPT],
                    rhs=seqb[k][:, 0:512],
                    start=(k == 0),
                    stop=(k == NCH - 1),
                )
            for k in range(NCH):
                nc.tensor.matmul(
                    out=psB[:],
                    lhsT=gs[k][:, lo : lo + PPT],
                    rhs=seqb[k][:, 512 : HS + 1],
                    start=(k == 0),
                    stop=(k == NCH - 1),
                )
            dsum = small_p.tile([PPT, 1], F32, name="dsum")
            nc.vector.tensor_scalar_add(out=dsum[:], in0=psB[:, 256:257], scalar1=1e-10)
            drec = small_p.tile([PPT, 1], F32, name="drec")
            nc.vector.reciprocal(out=drec[:], in_=dsum[:])
            st = rst_p.tile([PPT, HS], F32, name="st")
            nc.vector.tensor_scalar_mul(out=st[:, 0:512], in0=psA[:], scalar1=drec[:])
            nc.scalar.activation(
                out=st[:, 512:HS], in_=psB[:, 0:256],
                func=mybir.ActivationFunctionType.Copy, scale=drec[:],
            )
            nc.sync.dma_start(out=rs_out[lo : lo + PPT, :], in_=st[:])
            sts.append(st)

    nc.finalize()
    return nc


def _host_inputs(sequence_output, attention, entity_starts):
    """Build the 8 per-core input maps."""
    sel_np = np.zeros([PPT, 4 * NH], np.float32)
    for q in range(4):
        for p in range(PPT):
            sel_np[p, q * NH + q * PPT + p] = 0.25

    in_maps = []
    for cid in range(N_CORES):
        d, rho = cid // 2, cid % 2
        perm = (np.arange(NE) + rho * IL) % NE            # slot -> real entity
        starts_doc = np.asarray(entity_starts[d], dtype=np.int64)
        pstarts = starts_doc[perm]                        # [42, 4]
        pos = pstarts + OFFSET                            # mention positions, < 1024

        ia = np.zeros([PPT, 16], np.int32)
        for t in range(16):
            r, q = t // 4, t % 4
            p = np.arange(PPT)
            g = q * PPT + p
            n, h = g // H, g % H
            ia[:, t] = (h * C + pos[n, r]).astype(np.int32)

        iseq = pos.astype(np.int32)                       # [42, 4]

        att_doc = np.ascontiguousarray(
            np.asarray(attention[d], dtype=np.float32).reshape(H * C, C)
        )
        seq_doc = np.ascontiguousarray(np.asarray(sequence_output[d], dtype=np.float32))
        seq_lse = np.ascontiguousarray(seq_doc[:, rho * WLSE : (rho + 1) * WLSE])

        in_maps.append(
            {
                "att": att_doc,
                "seq": seq_doc,
                "seq_lse": seq_lse,
                "sel": sel_np,
                "idx_att": ia,
                "idx_seq": iseq,
            }
        )
    return in_maps


def _assemble(results, entity_starts, hts):
    eemb = np.empty([B, NE, HS], np.float32)
    rs_grid = np.empty([B, NE, NE, HS], np.float32)
    perm1 = (np.arange(NE) + IL) % NE
    for d in range(B):
        o0 = results[2 * d]["eemb_out"]
        o1 = results[2 * d + 1]["eemb_out"]
        eemb[d, :, 0:WLSE] = o0
        eemb[d, perm1, WLSE:HS] = o1

        g0 = results[2 * d]["rs_out"].reshape(IL, NE, HS)
        g1 = results[2 * d + 1]["rs_out"].reshape(IL, NE, HS)
        rs_grid[d, 0:IL] = g0
        tmp = np.empty([IL, NE, HS], np.float32)
        tmp[:, perm1] = g1
        rs_grid[d, IL:NE] = tmp

    hts_np = np.asarray(hts, dtype=np.int64)
    h_idx = hts_np[:, :, 0]
    t_idx = hts_np[:, :, 1]
    hss = np.empty([B, NE * NE, HS], np.float32)
    rss = np.empty([B, NE * NE, HS], np.float32)
    tss = np.empty([B, NE * NE, HS], np.float32)
    for d in range(B):
        hss[d] = eemb[d][h_idx[d]]
        tss[d] = eemb[d][t_idx[d]]
        rss[d] = rs_grid[d][h_idx[d], t_idx[d]]
    shape = (B, NE, NE, HS)
    return hss.reshape(shape), rss.reshape(shape), tss.reshape(shape)


def kernel(sequence_output, attention, entity_starts, hts):
    if "nc" not in _prog_cache:
        _prog_cache["nc"] = _build_program()
    nc = _prog_cache["nc"]

    in_maps = _host_inputs(sequence_output, attention, entity_starts)
    res = run_bass_kernel_spmd(nc, in_maps, list(range(N_CORES))).results
    return _assemble(res, entity_starts, hts)


if __name__ == "__main__":
    # smoke test with random data
    rng = np.random.default_rng(0)
    seq = rng.standard_normal((B, C, HS), dtype=np.float32)
    att = rng.random((B, H, C, C), dtype=np.float32)
    starts = rng.integers(0, 1020, (B, NE, M))
    hts = rng.integers(0, NE, (B, NE * NE, 2))
    outs = kernel(seq, att, starts, hts)
    print([o.shape for o in outs])


# revision 20
# speedup vs baseline: 1.1499x; 1.1201x over previous
"""Trainium2 Bass kernel for DocREModel_KD head (ragged_sequence).

Problem shape (hardcoded, per spec):
  sequence_output [4, 1024, 768] f32
  attention       [4, 12, 1024, 1024] f32
  entity_starts   [4, 42, 4] int
  hts             [4, 1764, 2] int
Outputs: (hss, rss, tss) each [4, 42, 42, 768] f32.

Strategy (8 cores, SPMD single program):
  - 2 cores per document. Core parity rho in {0,1}: the pair grid over the
    42x42 entity pairs is split by head-entity. The program always computes
    grid rows i in [0..20]; core rho=1 receives entity indices rotated by 21
    (host-side permutation of the tiny index tensors), so the same program
    computes the other half of the grid.
  - All data-dependent gathers use indirect DMA with host-computed row
    indices fed as int32 inputs (SPMD-safe).
  - e_att (mention-mean of attention rows) is computed c-partitioned by a
    single fused PE matmul against a constant selection matrix (mean +
    transpose in one step, f32r).
  - Pair grid G[c,(i,j)] = sum_h EA[c,i,h]*EA[c,j,h] via one broadcast-AP
    DVE product (bf16, 2x mode) + grouped tree reduction, relu'd.
  - rs = (relu(G) @ seq_aug) with an appended ones column giving the
    normalizer for free; normalization folded into the PSUM drain.
  - e_emb logsumexp is d-split across the core pair (rho chooses which half
    of the hidden dim), exp/ln on ScalarE.
  - hss/tss (pure row replications of e_emb) and the hts->grid-row mapping
    are assembled host-side from the device-computed e_emb / rs grid.
"""

import numpy as np
from contextlib import ExitStack

import concourse.bass as bass
import concourse.bacc as bacc
import concourse.mybir as mybir
import concourse.tile as tile
from concourse.bass_utils import run_bass_kernel_spmd

# ---- problem constants ----
B, H, C, HS, NE, M = 4, 12, 1024, 768, 42, 4
OFFSET = 1
NH = NE * H          # 504 (n,h) pairs
IL = NE // 2         # 21 grid rows per core
NB = 3               # i-blocks of 7 rows; block b covers j in [7b, 42)
BW = 7
BLKW = [NE - BW * b for b in range(NB)]        # 42, 35, 28
BLKOFF = [0, BW * BLKW[0], BW * (BLKW[0] + BLKW[1])]  # row offsets: 0, 294, 539
U = BW * sum(BLKW)   # 735 packed grid rows per core (canonical min<=max reps)
PPT = 126            # partitions per gathered RAW tile (504 = 4*126)
NCH = C // 128       # 8 c-chunks
WLSE = HS // 2       # 384: e_emb d-split width per core
N_CORES = 8

F32 = mybir.dt.float32
F32R = mybir.dt.float32r
BF16 = mybir.dt.bfloat16
I32 = mybir.dt.int32

_prog_cache = {}


def _build_program():
    nc = bacc.Bacc(None)

    att = nc.dram_tensor("att", [H * C, C], F32R, kind="ExternalInput")
    seq = nc.dram_tensor("seq", [C, HS], F32, kind="ExternalInput")
    seq_lse = nc.dram_tensor("seq_lse", [C, WLSE], F32, kind="ExternalInput")
    sel_d = nc.dram_tensor("sel", [PPT, 4 * NH], F32R, kind="ExternalInput")
    idx_att_d = nc.dram_tensor("idx_att", [PPT, 16], I32, kind="ExternalInput")
    idx_seq_d = nc.dram_tensor("idx_seq", [NE, M], I32, kind="ExternalInput")

    rs_out = nc.dram_tensor("rs_out", [U, HS], F32, kind="ExternalOutput")
    eemb_out = nc.dram_tensor("eemb_out", [NE, WLSE], F32, kind="ExternalOutput")

    with tile.TileContext(nc) as tc, ExitStack() as ctx:
        const_p = ctx.enter_context(tc.tile_pool(name="const", bufs=1))
        raw_p = ctx.enter_context(tc.tile_pool(name="raw", bufs=1))
        seqf_p = ctx.enter_context(tc.tile_pool(name="seqf", bufs=1))
        seqb_p = ctx.enter_context(tc.tile_pool(name="seqb", bufs=1))
        ea_p = ctx.enter_context(tc.tile_pool(name="ea", bufs=1))
        pr_p = ctx.enter_context(tc.tile_pool(name="pr", bufs=1))
        t4_p = ctx.enter_context(tc.tile_pool(name="t4", bufs=1))
        t2_p = ctx.enter_context(tc.tile_pool(name="t2", bufs=1))
        g_p = ctx.enter_context(tc.tile_pool(name="g", bufs=1))
        lse_p = ctx.enter_context(tc.tile_pool(name="lse", bufs=1))
        rst_p = ctx.enter_context(tc.tile_pool(name="rst", bufs=2))
        small_p = ctx.enter_context(tc.tile_pool(name="small", bufs=2))

        ea_ps = ctx.enter_context(tc.tile_pool(name="eaps", bufs=2, space="PSUM"))
        rsA_ps = ctx.enter_context(tc.tile_pool(name="rsA", bufs=3, space="PSUM"))
        rsB_ps = ctx.enter_context(tc.tile_pool(name="rsB", bufs=3, space="PSUM"))

        # --- constants / indices to SBUF ---
        sel_sb = const_p.tile([PPT, 4 * NH], F32R, name="sel_sb")
        nc.sync.dma_start(out=sel_sb[:], in_=sel_d[:])
        ia_sb = const_p.tile([PPT, 16], I32, name="ia_sb")
        nc.sync.dma_start(out=ia_sb[:], in_=idx_att_d[:])
        is_sb = const_p.tile([NE, M], I32, name="is_sb")
        nc.sync.dma_start(out=is_sb[:], in_=idx_seq_d[:])

        # --- indirect gathers: attention mention rows ---
        raws = []
        for t in range(16):
            rt = raw_p.tile([PPT, C], F32R, name=f"raw{t}")
            nc.gpsimd.indirect_dma_start(
                out=rt[:],
                out_offset=None,
                in_=att[:],
                in_offset=bass.IndirectOffsetOnAxis(ap=ia_sb[:, t : t + 1], axis=0),
            )
            raws.append(rt)

        # --- e_emb logsumexp pipeline (d-split half, exact fp32) ---
        sg = []
        for r in range(M):
            g = lse_p.tile([NE, WLSE], F32, name=f"sg{r}")
            nc.gpsimd.indirect_dma_start(
                out=g[:],
                out_offset=None,
                in_=seq_lse[:],
                in_offset=bass.IndirectOffsetOnAxis(ap=is_sb[:, r : r + 1], axis=0),
            )
            sg.append(g)
        ex = []
        for r in range(M):
            e = lse_p.tile([NE, WLSE], F32, name=f"ex{r}")
            nc.scalar.activation(out=e[:], in_=sg[r][:], func=mybir.ActivationFunctionType.Exp)
            ex.append(e)
        s01 = lse_p.tile([NE, WLSE], F32, name="s01")
        s23 = lse_p.tile([NE, WLSE], F32, name="s23")
        nc.vector.tensor_add(out=s01[:], in0=ex[0][:], in1=ex[1][:])
        nc.vector.tensor_add(out=s23[:], in0=ex[2][:], in1=ex[3][:])
        nc.vector.tensor_add(out=s01[:], in0=s01[:], in1=s23[:])
        lse_res = lse_p.tile([NE, WLSE], F32, name="lse_res")
        nc.scalar.activation(out=lse_res[:], in_=s01[:], func=mybir.ActivationFunctionType.Ln)
        # ACT-issued DMA: same-engine ordering after the Ln, so the DMA
        # carries only its ring-FIFO wait.
        nc.scalar.dma_start(out=eemb_out[:], in_=lse_res[:])

        # --- sequence chunks: load f32, convert to bf16, append ones col ---
        seqb = []
        for k in range(NCH):
            sf = seqf_p.tile([128, HS], F32, name=f"sf{k}")
            nc.sync.dma_start(out=sf[:], in_=seq[k * 128 : (k + 1) * 128, :])
            sb = seqb_p.tile([128, HS + 1], BF16, name=f"sb{k}")
            nc.scalar.copy(out=sb[:, 0:HS], in_=sf[:])
            nc.vector.memset(sb[:, HS : HS + 1], 1.0)
            seqb.append(sb)

        # --- EA: mention-mean + transpose via SEL matmul (f32r) ---
        eas = []
        for k in range(NCH):
            ps = ea_ps.tile([128, NH], F32, name="eaps")
            for t in range(16):
                q = t % 4
                nc.tensor.matmul(
                    out=ps[:],
                    lhsT=raws[t][:, k * 128 : (k + 1) * 128],
                    rhs=sel_sb[:, q * NH : (q + 1) * NH],
                    start=(t == 0),
                    stop=(t == 15),
                )
            ea = ea_p.tile([128, NH], BF16, name=f"ea{k}")
            nc.scalar.copy(out=ea[:], in_=ps[:])
            eas.append(ea)

        # --- pair-grid products + grouped h-reduction + relu ---
        gs = []
        for k in range(NCH):
            pr = pr_p.tile([128, U * H], BF16, name="pr")
            ea3 = eas[k][:].rearrange("p (i h) -> p i h", h=H)          # [128, 42, 12]
            for b in range(NB):
                w = BLKW[b]
                jf = BW * b
                in0 = ea3[:, jf : jf + BW, :].unsqueeze(2).to_broadcast([128, BW, w, H])
                in1 = ea3[:, jf:NE, :].unsqueeze(1).to_broadcast([128, BW, w, H])
                sec = pr[:, BLKOFF[b] * H : (BLKOFF[b] + BW * w) * H]
                pr4 = sec.rearrange("p (i j h) -> p i j h", j=w, h=H)
                nc.vector.tensor_tensor(out=pr4, in0=in0, in1=in1, op=mybir.AluOpType.mult)

            pru = pr[:].rearrange("p (u h) -> p u h", h=H)              # [128, U, 12]
            t4 = t4_p.tile([128, U * 4], BF16, name="t4")
            t4v = t4[:].rearrange("p (u f) -> p u f", f=4)
            nc.vector.tensor_tensor(out=t4v, in0=pru[:, :, 0:4], in1=pru[:, :, 4:8], op=mybir.AluOpType.add)
            nc.vector.tensor_tensor(out=t4v, in0=t4v, in1=pru[:, :, 8:12], op=mybir.AluOpType.add)
            t2 = t2_p.tile([128, U * 2], BF16, name="t2")
            t2v = t2[:].rearrange("p (u f) -> p u f", f=2)
            nc.vector.tensor_tensor(out=t2v, in0=t4v[:, :, 0:2], in1=t4v[:, :, 2:4], op=mybir.AluOpType.add)
            gp = g_p.tile([128, U], BF16, name=f"gp{k}")
            a = t2v[:, :, 0:1].squeeze(2)
            b = t2v[:, :, 1:2].squeeze(2)
            nc.vector.tensor_tensor(out=gp[:], in0=a, in1=b, op=mybir.AluOpType.add)
            g_t = g_p.tile([128, U], BF16, name=f"g{k}")
            nc.scalar.activation(out=g_t[:], in_=gp[:], func=mybir.ActivationFunctionType.Relu)
            gs.append(g_t)

        # --- rs matmul + fused normalization drain (drains on ACT so the
        # ACT-issued output DMA and the PSUM-bank-reuse waits stay single) ---
        ntau = (U + PPT - 1) // PPT                      # 6 (last tau: 105 rows)
        for tau in range(ntau):
            lo = tau * PPT
            rows = min(PPT, U - lo)
            psA = rsA_ps.tile([PPT, 512], F32, name="psA")
            psB = rsB_ps.tile([PPT, HS + 1 - 512], F32, name="psB")   # [126, 257]
            for k in range(NCH):
                nc.tensor.matmul(
                    out=psB[:rows],
                    lhsT=gs[k][:, lo : lo + rows],
                    rhs=seqb[k][:, 512 : HS + 1],
                    start=(k == 0),
                    stop=(k == NCH - 1),
                )
            dsum = small_p.tile([PPT, 1], F32, name="dsum")
            nc.vector.tensor_scalar_add(out=dsum[:rows], in0=psB[:rows, 256:257], scalar1=1e-10)
            drec = small_p.tile([PPT, 1], F32, name="drec")
            nc.vector.reciprocal(out=drec[:rows], in_=dsum[:rows])
            for k in range(NCH):
                nc.tensor.matmul(
                    out=psA[:rows],
                    lhsT=gs[k][:, lo : lo + rows],
                    rhs=seqb[k][:, 0:512],
                    start=(k == 0),
                    stop=(k == NCH - 1),
                )
            st = rst_p.tile([PPT, HS], F32, name="st")
            nc.vector.tensor_scalar_mul(out=st[:rows, 0:512], in0=psA[:rows], scalar1=drec[:rows])
            nc.scalar.activation(
                out=st[:rows, 512:HS], in_=psB[:rows, 0:256],
                func=mybir.ActivationFunctionType.Copy, scale=drec[:rows],
            )
            nc.sync.dma_start(out=rs_out[lo : lo + rows, :], in_=st[:rows])

    nc.finalize()
    return nc


def _host_inputs(sequence_output, attention, entity_starts):
    """Build the 8 per-core input maps."""
    sel_np = np.zeros([PPT, 4 * NH], np.float32)
    for q in range(4):
        for p in range(PPT):
            sel_np[p, q * NH + q * PPT + p] = 0.25

    in_maps = []
    for cid in range(N_CORES):
        d, rho = cid // 2, cid % 2
        perm = (np.arange(NE) + rho * IL) % NE            # slot -> real entity
        starts_doc = np.asarray(entity_starts[d], dtype=np.int64)
        pstarts = starts_doc[perm]                        # [42, 4]
        pos = pstarts + OFFSET                            # mention positions, < 1024

        ia = np.zeros([PPT, 16], np.int32)
        for t in range(16):
            r, q = t // 4, t % 4
            p = np.arange(PPT)
            g = q * PPT + p
            n, h = g // H, g % H
            ia[:, t] = (h * C + pos[n, r]).astype(np.int32)

        iseq = pos.astype(np.int32)                       # [42, 4]

        att_doc = np.ascontiguousarray(
            np.asarray(attention[d], dtype=np.float32).reshape(H * C, C)
        )
        seq_doc = np.ascontiguousarray(np.asarray(sequence_output[d], dtype=np.float32))
        seq_lse = np.ascontiguousarray(seq_doc[:, rho * WLSE : (rho + 1) * WLSE])

        in_maps.append(
            {
                "att": att_doc,
                "seq": seq_doc,
                "seq_lse": seq_lse,
                "sel": sel_np,
                "idx_att": ia,
                "idx_seq": iseq,
            }
        )
    return in_maps


_row_table_cache = {}


def _grid_row_table():
    if "t" not in _row_table_cache:
        row_of = np.full((IL, NE), -1, np.int64)
        for b in range(NB):
            w = BLKW[b]
            jf = BW * b
            for il in range(BW):
                for j in range(jf, NE):
                    row_of[BW * b + il, j] = BLKOFF[b] + il * w + (j - jf)
        _row_table_cache["t"] = row_of
    return _row_table_cache["t"]


def _assemble(results, entity_starts, hts):
    eemb = np.empty([B, NE, HS], np.float32)
    rs_grid = np.empty([B, NE, NE, HS], np.float32)
    perm1 = (np.arange(NE) + IL) % NE
    for d in range(B):
        o0 = results[2 * d]["eemb_out"]
        o1 = results[2 * d + 1]["eemb_out"]
        eemb[d, :, 0:WLSE] = o0
        eemb[d, perm1, WLSE:HS] = o1

        row_of = _grid_row_table()
        g0 = results[2 * d]["rs_out"]
        g1 = results[2 * d + 1]["rs_out"]
        # canonical representative (mn, mx); mn<=20 lives on the even core,
        # mn>=21 on the odd core at slots (mn-21, mx-21)
        for i in range(NE):
            for j in range(NE):
                mn, mx = (i, j) if i <= j else (j, i)
                if mn < IL:
                    rs_grid[d, i, j] = g0[row_of[mn, mx]]
                else:
                    rs_grid[d, i, j] = g1[row_of[mn - IL, mx - IL]]

    hts_np = np.asarray(hts, dtype=np.int64)
    h_idx = hts_np[:, :, 0]
    t_idx = hts_np[:, :, 1]
    hss = np.empty([B, NE * NE, HS], np.float32)
    rss = np.empty([B, NE * NE, HS], np.float32)
    tss = np.empty([B, NE * NE, HS], np.float32)
    for d in range(B):
        hss[d] = eemb[d][h_idx[d]]
        tss[d] = eemb[d][t_idx[d]]
        rss[d] = rs_grid[d][h_idx[d], t_idx[d]]
    shape = (B, NE, NE, HS)
    return hss.reshape(shape), rss.reshape(shape), tss.reshape(shape)


def kernel(sequence_output, attention, entity_starts, hts):
    if "nc" not in _prog_cache:
        _prog_cache["nc"] = _build_program()
    nc = _prog_cache["nc"]

    in_maps = _host_inputs(sequence_output, attention, entity_starts)
    res = run_bass_kernel_spmd(nc, in_maps, list(range(N_CORES))).results
    return _assemble(res, entity_starts, hts)


if __name__ == "__main__":
    # smoke test with random data
    rng = np.random.default_rng(0)
    seq = rng.standard_normal((B, C, HS), dtype=np.float32)
    att = rng.random((B, H, C, C), dtype=np.float32)
    starts = rng.integers(0, 1020, (B, NE, M))
    hts = rng.integers(0, NE, (B, NE * NE, 2))
    outs = kernel(seq, att, starts, hts)
    print([o.shape for o in outs])
